# revision 15
# baseline (speedup 1.0000x reference)
"""Cepstrum -> minimum-phase impulse response on 8 Trainium2 NeuronCores.

Math: the reference recurrence  n*h_n = sum_k (k c_k) h_{n-k}, h_0 = exp(c_0)
is exactly the power-series exponential h = exp(C(z)) mod z^512 for the
degree-255 polynomial C. We evaluate it spectrally:

    h = IDFT_L( exp( DFT_L(c) ) )[:512],  L = 768

which is exact up to aliasing of exp(C)'s tail beyond degree L (~2.4e-5 abs
here; validated offline against the recurrence in float64).

Device pipeline per 512-row supertile (all matmuls fp16 in / fp32 psum):
  - fwd:  R/I[bins, rows] = Wcos/Wsin-stationary matmuls over cT (bins 1..384)
  - ACT:  A = exp(R), Sn = sin2pi(I/2pi), Cs = sin2pi(I/2pi + 1/4)
          (single ACT table set: exp_and_friends = {exp, sin2pi})
  - DVE:  HRe = A*Cs, HIm = A*Sn
  - inv:  h[rows, n] = sum_bins HRe*ci + HIm*si  (H-stationary matmuls)
  - DVE:  out = h_psum + h0n (per-row DC term) + corr (fp16-rounding
          compensation column, see _host_weights), DMA out.

Host-side input marshaling (not device work): shard rows across 8 cores,
pre-transpose c to (256, rows) fp16 so the DFT contraction dim lands on SBUF
partitions without on-device transposes, and precompute the DC bin's scalar
h0n = exp(sum_d c_d)/L per row (0.008% of the FLOPs).
"""

import os
import sys
from contextlib import ExitStack

import numpy as np

for _p in ("/opt/trn_rl_repo", "/root/.axon_site/_ro/trn_rl_repo"):
    if os.path.isdir(_p) and _p not in sys.path:
        sys.path.insert(0, _p)

from concourse import bacc, mybir, tile  # noqa: E402
from concourse.bass_utils import run_bass_kernel_spmd  # noqa: E402

B_TOTAL = 131072
N_CORES = 8
B_CORE = B_TOTAL // N_CORES  # 16384
M1 = 256          # cepstral coefficients per row (M+1)
N_OUT = 512       # impulse response length
L = 768           # DFT length (aliasing ~2.4e-5 abs, validated offline)
NB = L // 2       # matmul-handled bins 1..NB (bin 0 handled via h0n)
NBC = NB // 128   # bin chunks
ST_ROWS = 512     # rows per supertile
N_ST = B_CORE // ST_ROWS  # 32

F32 = mybir.dt.float32
F16 = mybir.dt.float16

_cache: dict = {}

TWO_PI = 2.0 * np.pi


def _install_sin2pi_patches():
    """Keep all activations in ONE ACT table set (exp_and_friends = {exp,
    sin2pi}) to avoid per-supertile table reloads (~2.7us each).

    1. Patch bacc's activation-table map so Exp and Sin both resolve to
       exp_and_friends -> bacc emits a single LoadActFuncSet.
    2. Rewrite "Sin" -> "Sin2pi" in the BIR json just before walrus; the
       kernel emits Sin with scale=1/(2*pi) so the arguments are already
       in sin2pi's convention (sin2pi(x) = sin(2*pi*x)).
    """
    if _cache.get("patched"):
        return
    import concourse.bacc as _bacc
    import concourse.bass2jax as _b2j

    SIN = mybir.ActivationFunctionType.Sin
    EXP = mybir.ActivationFunctionType.Exp
    _orig_tables = _bacc.get_activation_tables

    def tables_patched(arch):
        t = {k: set(v) for k, v in _orig_tables(arch).items()}
        for k in t:
            t[k].discard(SIN)
            if k != "exp_and_friends":
                t[k].discard(EXP)
        if "exp_and_friends" in t:
            t["exp_and_friends"] |= {SIN, EXP}
        return t

    _bacc.get_activation_tables = tables_patched

    _orig_compile = _b2j.compile_bir_kernel

    def compile_patched(bir_json, *a, **kw):
        if isinstance(bir_json, bytes):
            bir_json = bir_json.replace(b'"func":"Sin"', b'"func":"Sin2pi"')
        else:
            bir_json = bir_json.replace('"func":"Sin"', '"func":"Sin2pi"')
        return _orig_compile(bir_json, *a, **kw)

    _b2j.compile_bir_kernel = compile_patched
    _cache["patched"] = True


def _host_weights():
    d = np.arange(M1, dtype=np.float64)
    k = np.arange(1, NB + 1, dtype=np.float64)
    th = 2.0 * np.pi * np.outer(d, k) / L           # (256, NB)
    wc = np.cos(th)
    ws = -np.sin(th)
    n = np.arange(N_OUT, dtype=np.float64)
    thi = 2.0 * np.pi * np.outer(k, n) / L          # (NB, 512)
    w = np.where(k == NB, 1.0, 2.0)[:, None] / L
    ci = w * np.cos(thi)
    si = -w * np.sin(thi)
    ci16 = ci.astype(np.float16)
    si16 = si.astype(np.float16)
    # Coherent part of the fp16 rounding error of ci: the H ~= 1 background
    # does not cancel it (validated offline). Subtract column sums on-device.
    corr = -(ci16.astype(np.float64) - ci).sum(0)          # (512,)
    corr128 = np.broadcast_to(corr.astype(np.float32), (128, N_OUT)).copy()
    return (
        wc.astype(np.float16),
        ws.astype(np.float16),
        ci16,
        si16,
        corr128,
    )


def _build(n_st=N_ST, repeat=1):
    _install_sin2pi_patches()
    nc = bacc.Bacc(
        "TRN2", target_bir_lowering=False, debug=False, num_devices=N_CORES
    )
    ct_ap = nc.dram_tensor("ct", [M1, n_st * ST_ROWS], F16, kind="ExternalInput").ap()
    h0_ap = nc.dram_tensor("h0n", [n_st, 128, 4], F32, kind="ExternalInput").ap()
    wc_ap = nc.dram_tensor("wcf", [M1, NB], F16, kind="ExternalInput").ap()
    ws_ap = nc.dram_tensor("wsf", [M1, NB], F16, kind="ExternalInput").ap()
    ci_ap = nc.dram_tensor("cif", [NB, N_OUT], F16, kind="ExternalInput").ap()
    si_ap = nc.dram_tensor("sif", [NB, N_OUT], F16, kind="ExternalInput").ap()
    corr_ap = nc.dram_tensor("corrf", [128, N_OUT], F32, kind="ExternalInput").ap()
    h_ap = nc.dram_tensor("h", [n_st * ST_ROWS, N_OUT], F32, kind="ExternalOutput").ap()

    EXP = mybir.ActivationFunctionType.Exp
    SIN = mybir.ActivationFunctionType.Sin

    with tile.TileContext(nc) as tc, ExitStack() as ctx:
        const = ctx.enter_context(tc.tile_pool(name="const", bufs=1))
        ctp = ctx.enter_context(tc.tile_pool(name="ctp", bufs=6))
        actp = ctx.enter_context(tc.tile_pool(name="actp", bufs=4))
        hp = ctx.enter_context(tc.tile_pool(name="hp", bufs=12))
        outp = ctx.enter_context(tc.tile_pool(name="outp", bufs=6))
        dcp = ctx.enter_context(tc.tile_pool(name="dcp", bufs=4))
        ps_ri = ctx.enter_context(tc.tile_pool(name="ps_ri", bufs=3, space="PSUM"))
        ps_h = ctx.enter_context(tc.tile_pool(name="ps_h", bufs=2, space="PSUM"))

        # constants
        wc_sb = [const.tile([128, NB], F16, tag=f"wc{d}", name=f"wc{d}") for d in range(2)]
        ws_sb = [const.tile([128, NB], F16, tag=f"ws{d}", name=f"ws{d}") for d in range(2)]
        for d in range(2):
            nc.sync.dma_start(wc_sb[d][:], wc_ap[d * 128:(d + 1) * 128, :])
            nc.sync.dma_start(ws_sb[d][:], ws_ap[d * 128:(d + 1) * 128, :])
        ci_sb = [const.tile([128, N_OUT], F16, tag=f"ci{b}", name=f"ci{b}") for b in range(NBC)]
        si_sb = [const.tile([128, N_OUT], F16, tag=f"si{b}", name=f"si{b}") for b in range(NBC)]
        for b in range(NBC):
            nc.sync.dma_start(ci_sb[b][:], ci_ap[b * 128:(b + 1) * 128, :])
            nc.sync.dma_start(si_sb[b][:], si_ap[b * 128:(b + 1) * 128, :])
        corr_sb = const.tile([128, N_OUT], F32, tag="corr_sb")
        nc.sync.dma_start(corr_sb[:], corr_ap[:])
        zb = const.tile([128, 1], F32, tag="zb")
        nc.gpsimd.memset(zb[:], 0.0)
        quarter = const.tile([128, 1], F32, tag="quarter")
        nc.gpsimd.memset(quarter[:], 0.25)

        for st in range(n_st * repeat):
            st = st % n_st
            r0 = st * ST_ROWS
            # ---- load pre-transposed cT chunks and DC scalars ----
            cT16 = []
            for d in range(2):
                t = ctp.tile([128, ST_ROWS], F16, tag="cT16", name="cT16")
                nc.sync.dma_start(t[:], ct_ap[d * 128:(d + 1) * 128, r0:r0 + ST_ROWS])
                cT16.append(t)
            h0n = dcp.tile([128, 4], F32, tag="h0n")
            nc.sync.dma_start(h0n[:], h0_ap[st])

            # ---- forward DFT + pointwise, per bin-chunk ----
            HRe = []
            HIm = []
            for bc in range(NBC):
                r_ps = ps_ri.tile([128, ST_ROWS], F32, tag="R")
                i_ps = ps_ri.tile([128, ST_ROWS], F32, tag="I")
                for d in range(2):
                    nc.tensor.matmul(
                        r_ps[:], wc_sb[d][:, bc * 128:(bc + 1) * 128], cT16[d][:],
                        start=(d == 0), stop=(d == 1),
                    )
                for d in range(2):
                    nc.tensor.matmul(
                        i_ps[:], ws_sb[d][:, bc * 128:(bc + 1) * 128], cT16[d][:],
                        start=(d == 0), stop=(d == 1),
                    )
                a_sb = actp.tile([128, ST_ROWS], F16, tag="A")
                sn_sb = actp.tile([128, ST_ROWS], F16, tag="Sn")
                cs_sb = actp.tile([128, ST_ROWS], F16, tag="Cs")
                nc.scalar.activation(a_sb[:], r_ps[:], EXP, bias=zb[:])
                nc.scalar.activation(sn_sb[:], i_ps[:], SIN, bias=zb[:],
                                     scale=float(1.0 / TWO_PI))
                nc.scalar.activation(cs_sb[:], i_ps[:], SIN, bias=quarter[:],
                                     scale=float(1.0 / TWO_PI))
                hre = hp.tile([128, ST_ROWS], F16, tag="HRe")
                him = hp.tile([128, ST_ROWS], F16, tag="HIm")
                nc.vector.tensor_mul(hre[:], a_sb[:], cs_sb[:])
                nc.vector.tensor_mul(him[:], a_sb[:], sn_sb[:])
                HRe.append(hre)
                HIm.append(him)

            # ---- inverse DFT per row-chunk + DC/corr add + store ----
            for rc in range(4):
                h_ps = ps_h.tile([128, N_OUT], F32, tag="h_ps")
                for bc in range(NBC):
                    nc.tensor.matmul(
                        h_ps[:], HRe[bc][:, rc * 128:(rc + 1) * 128], ci_sb[bc][:],
                        start=(bc == 0), stop=False,
                    )
                    nc.tensor.matmul(
                        h_ps[:], HIm[bc][:, rc * 128:(rc + 1) * 128], si_sb[bc][:],
                        start=False, stop=(bc == NBC - 1),
                    )
                o_sb = outp.tile([128, N_OUT], F32, tag="o_sb")
                nc.vector.scalar_tensor_tensor(
                    o_sb[:], h_ps[:], h0n[:, rc:rc + 1], corr_sb[:],
                    op0=mybir.AluOpType.add, op1=mybir.AluOpType.add,
                )
                nc.sync.dma_start(
                    h_ap[r0 + rc * 128: r0 + (rc + 1) * 128, :], o_sb[:]
                )

    nc.compile()
    return nc


def _get_nc(n_st=N_ST):
    key = ("nc", n_st)
    if key not in _cache:
        _cache[key] = _build(n_st)
    return _cache[key]


def _marshal(c_shard):
    """Host-side input marshaling for one core's row shard."""
    ct = np.ascontiguousarray(c_shard.astype(np.float16).T)       # (256, rows)
    s0 = c_shard.astype(np.float64).sum(axis=1)                    # (rows,)
    h0n = (np.exp(s0) / L).astype(np.float32)
    n_st = c_shard.shape[0] // ST_ROWS
    # pack so tile [p, rc] = h0n[st*512 + rc*128 + p]
    h0n = h0n.reshape(n_st, 4, 128).transpose(0, 2, 1).copy()      # (n_st,128,4)
    return ct, h0n


def _in_maps(c):
    wc, ws, ci, si, corr = _host_weights()
    maps = []
    for i in range(N_CORES):
        ct, h0n = _marshal(c[i * B_CORE:(i + 1) * B_CORE])
        maps.append({
            "ct": ct, "h0n": h0n,
            "wcf": wc, "wsf": ws, "cif": ci, "sif": si, "corrf": corr,
        })
    return maps


def kernel(c):
    c = np.ascontiguousarray(np.asarray(c), dtype=np.float32)
    assert c.shape == (B_TOTAL, M1), c.shape
    nc = _get_nc()
    res = run_bass_kernel_spmd(nc, _in_maps(c), list(range(N_CORES)))
    return np.concatenate(
        [res.results[i]["h"] for i in range(N_CORES)], axis=0
    )


# revision 16
# speedup vs baseline: 295.2645x; 295.2645x over previous
"""Cepstrum -> minimum-phase impulse response on 8 Trainium2 NeuronCores.

Math: the reference recurrence  n*h_n = sum_k (k c_k) h_{n-k}, h_0 = exp(c_0)
is exactly the power-series exponential h = exp(C(z)) mod z^512 for the
degree-255 polynomial C. We evaluate it spectrally:

    h = IDFT_L( exp( DFT_L(c) ) )[:512],  L = 768

which is exact up to aliasing of exp(C)'s tail beyond degree L (~2.4e-5 abs
here; validated offline against the recurrence in float64).

Device pipeline per 512-row supertile (all matmuls fp16 in / fp32 psum):
  - fwd:  R/I[bins, rows] = Wcos/Wsin-stationary matmuls over cT (bins 1..384)
  - ACT:  A = exp(R), Sn = sin2pi(I/2pi), Cs = sin2pi(I/2pi + 1/4)
          (single ACT table set: exp_and_friends = {exp, sin2pi})
  - DVE:  HRe = A*Cs, HIm = A*Sn
  - inv:  h[rows, n] = sum_bins HRe*ci + HIm*si  (H-stationary matmuls)
  - DVE:  out = h_psum + h0n (per-row DC term) + corr (fp16-rounding
          compensation column, see _host_weights), DMA out.

Host-side input marshaling (not device work): shard rows across 8 cores,
pre-transpose c to (256, rows) fp16 so the DFT contraction dim lands on SBUF
partitions without on-device transposes, and precompute the DC bin's scalar
h0n = exp(sum_d c_d)/L per row (0.008% of the FLOPs).
"""

import os
import sys
from contextlib import ExitStack

import numpy as np

for _p in ("/opt/trn_rl_repo", "/root/.axon_site/_ro/trn_rl_repo"):
    if os.path.isdir(_p) and _p not in sys.path:
        sys.path.insert(0, _p)

from concourse import bacc, mybir, tile  # noqa: E402
from concourse.bass_utils import run_bass_kernel_spmd  # noqa: E402

B_TOTAL = 131072
N_CORES = 8
B_CORE = B_TOTAL // N_CORES  # 16384
M1 = 256          # cepstral coefficients per row (M+1)
N_OUT = 512       # impulse response length
L = 768           # DFT length (aliasing ~2.4e-5 abs, validated offline)
NB = L // 2       # matmul-handled bins 1..NB (bin 0 handled via h0n)
NBC = NB // 128   # bin chunks
ST_ROWS = 512     # rows per supertile
N_ST = B_CORE // ST_ROWS  # 32

F32 = mybir.dt.float32
F16 = mybir.dt.float16

_cache: dict = {}

TWO_PI = 2.0 * np.pi


def _install_sin2pi_patches():
    """Keep all activations in ONE ACT table set (exp_and_friends = {exp,
    sin2pi}) to avoid per-supertile table reloads (~2.7us each).

    1. Patch bacc's activation-table map so Exp and Sin both resolve to
       exp_and_friends -> bacc emits a single LoadActFuncSet.
    2. Rewrite "Sin" -> "Sin2pi" in the BIR json just before walrus; the
       kernel emits Sin with scale=1/(2*pi) so the arguments are already
       in sin2pi's convention (sin2pi(x) = sin(2*pi*x)).
    """
    if _cache.get("patched"):
        return
    import concourse.bacc as _bacc
    import concourse.bass2jax as _b2j

    SIN = mybir.ActivationFunctionType.Sin
    EXP = mybir.ActivationFunctionType.Exp
    _orig_tables = _bacc.get_activation_tables

    def tables_patched(arch):
        t = {k: set(v) for k, v in _orig_tables(arch).items()}
        for k in t:
            t[k].discard(SIN)
            if k != "exp_and_friends":
                t[k].discard(EXP)
        if "exp_and_friends" in t:
            t["exp_and_friends"] |= {SIN, EXP}
        return t

    _bacc.get_activation_tables = tables_patched

    _orig_compile = _b2j.compile_bir_kernel

    def compile_patched(bir_json, *a, **kw):
        # only rewrite THIS kernel's module (identified by its weight tensor)
        if isinstance(bir_json, bytes):
            if b'"wcf"' in bir_json:
                bir_json = bir_json.replace(b'"func":"Sin"', b'"func":"Sin2pi"')
        elif '"wcf"' in bir_json:
            bir_json = bir_json.replace('"func":"Sin"', '"func":"Sin2pi"')
        return _orig_compile(bir_json, *a, **kw)

    _b2j.compile_bir_kernel = compile_patched
    _cache["patched"] = True


def _host_weights():
    d = np.arange(M1, dtype=np.float64)
    k = np.arange(1, NB + 1, dtype=np.float64)
    th = 2.0 * np.pi * np.outer(d, k) / L           # (256, NB)
    wc = np.cos(th)
    ws = -np.sin(th)
    n = np.arange(N_OUT, dtype=np.float64)
    thi = 2.0 * np.pi * np.outer(k, n) / L          # (NB, 512)
    w = np.where(k == NB, 1.0, 2.0)[:, None] / L
    ci = w * np.cos(thi)
    si = -w * np.sin(thi)
    ci16 = ci.astype(np.float16)
    si16 = si.astype(np.float16)
    # Coherent part of the fp16 rounding error of ci: the H ~= 1 background
    # does not cancel it (validated offline). Subtract column sums on-device.
    corr = -(ci16.astype(np.float64) - ci).sum(0)          # (512,)
    corr128 = np.broadcast_to(corr.astype(np.float32), (128, N_OUT)).copy()
    return (
        wc.astype(np.float16),
        ws.astype(np.float16),
        ci16,
        si16,
        corr128,
    )


def _build(n_st=N_ST, repeat=1):
    _install_sin2pi_patches()
    nc = bacc.Bacc(
        "TRN2", target_bir_lowering=False, debug=False, num_devices=N_CORES
    )
    ct_ap = nc.dram_tensor("ct", [M1, n_st * ST_ROWS], F16, kind="ExternalInput").ap()
    h0_ap = nc.dram_tensor("h0n", [n_st, 128, 4], F32, kind="ExternalInput").ap()
    wc_ap = nc.dram_tensor("wcf", [M1, NB], F16, kind="ExternalInput").ap()
    ws_ap = nc.dram_tensor("wsf", [M1, NB], F16, kind="ExternalInput").ap()
    ci_ap = nc.dram_tensor("cif", [NB, N_OUT], F16, kind="ExternalInput").ap()
    si_ap = nc.dram_tensor("sif", [NB, N_OUT], F16, kind="ExternalInput").ap()
    corr_ap = nc.dram_tensor("corrf", [128, N_OUT], F32, kind="ExternalInput").ap()
    h_ap = nc.dram_tensor("h", [n_st * ST_ROWS, N_OUT], F32, kind="ExternalOutput").ap()

    EXP = mybir.ActivationFunctionType.Exp
    SIN = mybir.ActivationFunctionType.Sin

    with tile.TileContext(nc) as tc, ExitStack() as ctx:
        const = ctx.enter_context(tc.tile_pool(name="const", bufs=1))
        ctp = ctx.enter_context(tc.tile_pool(name="ctp", bufs=6))
        actp = ctx.enter_context(tc.tile_pool(name="actp", bufs=4))
        hp = ctx.enter_context(tc.tile_pool(name="hp", bufs=12))
        outp = ctx.enter_context(tc.tile_pool(name="outp", bufs=6))
        dcp = ctx.enter_context(tc.tile_pool(name="dcp", bufs=4))
        ps_ri = ctx.enter_context(tc.tile_pool(name="ps_ri", bufs=3, space="PSUM"))
        ps_h = ctx.enter_context(tc.tile_pool(name="ps_h", bufs=2, space="PSUM"))

        # constants
        wc_sb = [const.tile([128, NB], F16, tag=f"wc{d}", name=f"wc{d}") for d in range(2)]
        ws_sb = [const.tile([128, NB], F16, tag=f"ws{d}", name=f"ws{d}") for d in range(2)]
        for d in range(2):
            nc.sync.dma_start(wc_sb[d][:], wc_ap[d * 128:(d + 1) * 128, :])
            nc.sync.dma_start(ws_sb[d][:], ws_ap[d * 128:(d + 1) * 128, :])
        ci_sb = [const.tile([128, N_OUT], F16, tag=f"ci{b}", name=f"ci{b}") for b in range(NBC)]
        si_sb = [const.tile([128, N_OUT], F16, tag=f"si{b}", name=f"si{b}") for b in range(NBC)]
        for b in range(NBC):
            nc.sync.dma_start(ci_sb[b][:], ci_ap[b * 128:(b + 1) * 128, :])
            nc.sync.dma_start(si_sb[b][:], si_ap[b * 128:(b + 1) * 128, :])
        corr_sb = const.tile([128, N_OUT], F32, tag="corr_sb")
        nc.sync.dma_start(corr_sb[:], corr_ap[:])
        zb = const.tile([128, 1], F32, tag="zb")
        nc.gpsimd.memset(zb[:], 0.0)
        quarter = const.tile([128, 1], F32, tag="quarter")
        nc.gpsimd.memset(quarter[:], 0.25)

        for st in range(n_st * repeat):
            st = st % n_st
            r0 = st * ST_ROWS
            # ---- load pre-transposed cT chunks and DC scalars ----
            cT16 = []
            for d in range(2):
                t = ctp.tile([128, ST_ROWS], F16, tag="cT16", name="cT16")
                nc.sync.dma_start(t[:], ct_ap[d * 128:(d + 1) * 128, r0:r0 + ST_ROWS])
                cT16.append(t)
            h0n = dcp.tile([128, 4], F32, tag="h0n")
            nc.sync.dma_start(h0n[:], h0_ap[st])

            # ---- forward DFT + pointwise, per bin-chunk ----
            HRe = []
            HIm = []
            for bc in range(NBC):
                r_ps = ps_ri.tile([128, ST_ROWS], F32, tag="R")
                i_ps = ps_ri.tile([128, ST_ROWS], F32, tag="I")
                for d in range(2):
                    nc.tensor.matmul(
                        r_ps[:], wc_sb[d][:, bc * 128:(bc + 1) * 128], cT16[d][:],
                        start=(d == 0), stop=(d == 1),
                    )
                for d in range(2):
                    nc.tensor.matmul(
                        i_ps[:], ws_sb[d][:, bc * 128:(bc + 1) * 128], cT16[d][:],
                        start=(d == 0), stop=(d == 1),
                    )
                a_sb = actp.tile([128, ST_ROWS], F32, tag="A")
                sn_sb = actp.tile([128, ST_ROWS], F32, tag="Sn")
                cs_sb = actp.tile([128, ST_ROWS], F32, tag="Cs")
                nc.scalar.activation(a_sb[:], r_ps[:], EXP, bias=zb[:])
                nc.scalar.activation(sn_sb[:], i_ps[:], SIN, bias=zb[:],
                                     scale=float(1.0 / TWO_PI))
                nc.scalar.activation(cs_sb[:], i_ps[:], SIN, bias=quarter[:],
                                     scale=float(1.0 / TWO_PI))
                hre = hp.tile([128, ST_ROWS], F16, tag="HRe")
                him = hp.tile([128, ST_ROWS], F16, tag="HIm")
                nc.vector.tensor_mul(hre[:], a_sb[:], cs_sb[:])
                nc.vector.tensor_mul(him[:], a_sb[:], sn_sb[:])
                HRe.append(hre)
                HIm.append(him)

            # ---- inverse DFT per row-chunk + DC/corr add + store ----
            for rc in range(4):
                h_ps = ps_h.tile([128, N_OUT], F32, tag="h_ps")
                for bc in range(NBC):
                    nc.tensor.matmul(
                        h_ps[:], HRe[bc][:, rc * 128:(rc + 1) * 128], ci_sb[bc][:],
                        start=(bc == 0), stop=False,
                    )
                    nc.tensor.matmul(
                        h_ps[:], HIm[bc][:, rc * 128:(rc + 1) * 128], si_sb[bc][:],
                        start=False, stop=(bc == NBC - 1),
                    )
                o_sb = outp.tile([128, N_OUT], F32, tag="o_sb")
                nc.vector.scalar_tensor_tensor(
                    o_sb[:], h_ps[:], h0n[:, rc:rc + 1], corr_sb[:],
                    op0=mybir.AluOpType.add, op1=mybir.AluOpType.add,
                )
                nc.sync.dma_start(
                    h_ap[r0 + rc * 128: r0 + (rc + 1) * 128, :], o_sb[:]
                )

    nc.compile()
    return nc


def _get_nc(n_st=N_ST):
    key = ("nc", n_st)
    if key not in _cache:
        _cache[key] = _build(n_st)
    return _cache[key]


def _marshal(c_shard):
    """Host-side input marshaling for one core's row shard."""
    ct = np.ascontiguousarray(c_shard.astype(np.float16).T)       # (256, rows)
    s0 = c_shard.astype(np.float64).sum(axis=1)                    # (rows,)
    h0n = (np.exp(s0) / L).astype(np.float32)
    n_st = c_shard.shape[0] // ST_ROWS
    # pack so tile [p, rc] = h0n[st*512 + rc*128 + p]
    h0n = h0n.reshape(n_st, 4, 128).transpose(0, 2, 1).copy()      # (n_st,128,4)
    return ct, h0n


def _in_maps(c):
    wc, ws, ci, si, corr = _host_weights()
    maps = []
    for i in range(N_CORES):
        ct, h0n = _marshal(c[i * B_CORE:(i + 1) * B_CORE])
        maps.append({
            "ct": ct, "h0n": h0n,
            "wcf": wc, "wsf": ws, "cif": ci, "sif": si, "corrf": corr,
        })
    return maps


def kernel(c):
    c = np.ascontiguousarray(np.asarray(c), dtype=np.float32)
    assert c.shape == (B_TOTAL, M1), c.shape
    nc = _get_nc()
    res = run_bass_kernel_spmd(nc, _in_maps(c), list(range(N_CORES)))
    return np.concatenate(
        [res.results[i]["h"] for i in range(N_CORES)], axis=0
    )
